# revision 2
# baseline (speedup 1.0000x reference)
"""Trainium2 Bass kernel for nn_Discourse (BERT span-pool + 2x TransformerConv GNN).

Sharding: data-parallel over docs for the span pooling (8 docs/core);
head-parallel for the graph convs (head h on core h; C=1024 = 8 heads x 128).
Two on-device AllGathers move node features between the two shardings.

Program structure for get_program(ntpb, repeat): each compute segment
(pooling / layer1 / layer2+final) is wrapped in a hardware For_i(0, repeat)
loop, and the two AllGathers are emitted straight-line once per rep (CC
inside a hardware loop wedges NRT's exec unit). The rep=1 and rep=2
programs are therefore byte-identical except loop bounds + one extra pair
of CC instructions, so the harness's repeat-delta measures the marginal
execution cost of one full body (compute + collectives), not NEFF size.
Weight staging (SBUF loads of W*, edge tables, wvec precompute) is
loop-invariant and hoisted, as in the original kernel.

Key algebraic simplification: edge_attr @ eW is rank-2 in the edge scalars
(type, is_main):  e_vec(g) = t_g*wr + m_g*wm + wc  with wr = Wr @ eW,
wm = Wm @ eW, wc = (br+bm) @ eW (computed on device).  The k-side edge term
folds into per-node scalars A=q.wr, B=q.wm, C=q.wc; the v-side term folds
into three rank-1 outer products per node block.

Per-edge work uses dst-sorted edges bucketed by 128-node blocks:
dma_gather of node rows (dst payload [q|C|A|B], k rows, v rows), per-edge
logits = (q.k + t*A + m*B + C)/sqrt(d), exp without max-subtraction (|logit|
< 0.1 at this model scale; b2 is dropped since softmax is shift-invariant),
then scatter-softmax/aggregation via one-hot (iota==dst_local)*ex matmuls
accumulated in PSUM.
"""

import math

import numpy as np

import concourse.bacc as bacc
import concourse.bass as bass
import concourse.mybir as mybir
import concourse.tile as tile
from concourse.bass_utils import run_bass_kernel_spmd
from concourse.masks import make_identity

F32 = mybir.dt.float32
F32R = mybir.dt.float32r
BF16 = mybir.dt.bfloat16
I16 = mybir.dt.int16
I32 = mybir.dt.int32

B, E, L, D = 64, 512, 768, 0  # placeholder overwritten below
B, E, L, D = 64, 32, 512, 768
C1 = C2 = 1024
H = 8
EG = 32768
SPAN_H = 512
NLAB = 4
N = B * E            # 2048 nodes
NCORES = 8
DPC = B // NCORES    # 8 docs per core
NPC = N // NCORES    # 256 nodes per core (pooling side)
NBLK = N // 128      # 16 node blocks
ISD = 1.0 / math.sqrt(128.0)

AluOp = mybir.AluOpType
ActFn = mybir.ActivationFunctionType


def _wrap_idx(idx, pad_to=None):
    """int16 indices -> [128, n/16] wrapped layout (g at [g%16, g//16]),
    replicated across the 8 gpsimd cores (partition groups of 16)."""
    idx = np.asarray(idx, dtype=np.int16)
    n = idx.shape[0]
    if pad_to is not None and n < pad_to:
        idx = np.concatenate([idx, np.zeros(pad_to - n, np.int16)])
        n = pad_to
    assert n % 16 == 0
    w = idx.reshape(n // 16, 16).T
    return np.tile(w, (8, 1)).copy()


def _edge_prep(batch_edge, batch_edge_type, batch_is_main, ntpb):
    """Sort edges by dst, bucket into 16 blocks of 128 dst nodes, pad each
    block to ntpb tiles of 128 edges. Edge slot s of block b sits at
    [partition s%128, tile b*ntpb + s//128]."""
    src = np.asarray(batch_edge[0], dtype=np.int64)
    dst = np.asarray(batch_edge[1], dtype=np.int64)
    t = np.asarray(batch_edge_type, dtype=np.float32).reshape(-1)
    m = np.asarray(batch_is_main, dtype=np.float32).reshape(-1)

    order = np.argsort(dst, kind="stable")
    src, dst, t, m = src[order], dst[order], t[order], m[order]
    blk = dst // 128

    NT = NBLK * ntpb
    cap = ntpb * 128
    dstloc = np.zeros((128, NT), np.float32)
    tval = np.zeros((128, NT), np.float32)
    mval = np.zeros((128, NT), np.float32)
    vadd = np.full((128, NT), -100000.0, np.float32)
    ets = np.zeros((128, NT, 4), np.float32)
    srcidx_w = np.zeros((128, NT * 8), np.int16)
    dstloc_w = np.zeros((128, NT * 8), np.int16)

    for b in range(NBLK):
        sel = blk == b
        nb = int(sel.sum())
        assert nb <= cap, f"block {b}: {nb} edges > capacity {cap}"
        sl = np.zeros(cap, np.float32)
        sv = np.zeros(cap, np.int16)
        tv = np.zeros(cap, np.float32)
        mv = np.zeros(cap, np.float32)
        dl16 = np.zeros(cap, np.int16)
        sl[:nb] = (dst[sel] - b * 128).astype(np.float32)
        dl16[:nb] = (dst[sel] - b * 128).astype(np.int16)
        sv[:nb] = src[sel].astype(np.int16)
        tv[:nb] = t[sel]
        mv[:nb] = m[sel]
        va = np.full(cap, -100000.0, np.float32)
        va[:nb] = 0.0
        cols = slice(b * ntpb, (b + 1) * ntpb)
        dstloc[:, cols] = sl.reshape(ntpb, 128).T
        tval[:, cols] = tv.reshape(ntpb, 128).T
        mval[:, cols] = mv.reshape(ntpb, 128).T
        vadd[:, cols] = va.reshape(ntpb, 128).T
        ets[:, cols, 0] = 1.0
        ets[:, cols, 1] = tv.reshape(ntpb, 128).T
        ets[:, cols, 2] = mv.reshape(ntpb, 128).T
        srcidx_w[:, b * ntpb * 8:(b + 1) * ntpb * 8] = _wrap_idx(sv)
        dstloc_w[:, b * ntpb * 8:(b + 1) * ntpb * 8] = _wrap_idx(dl16)

    return dict(DSTLOC=dstloc, TVAL=tval, MVAL=mval, VADD=vadd, ETS=ets,
                SRCIDX=srcidx_w, DSTIDX=dstloc_w)


# --------------------------------------------------------------------------
# device program
# --------------------------------------------------------------------------

def build_program(ntpb, repeat=1):
    nc = bacc.Bacc("TRN2", target_bir_lowering=False, debug=False)
    NT = NBLK * ntpb

    def din(name, shape, dt):
        return nc.dram_tensor(name, shape, dt, kind="ExternalInput")

    io = dict(
        LHT=din("LHT", [D, DPC * L], F32R),
        LH=din("LH", [DPC * L, D], F32R),
        MASKT=din("MASKT", [E, DPC, L], F32),
        W1=din("W1", [D, SPAN_H], F32R),
        B1C=din("B1C", [128, 4], F32),
        W2=din("W2", [SPAN_H, 1], F32R),
        DSTLOC=din("DSTLOC", [128, NT], F32),
        TVAL=din("TVAL", [128, NT], F32),
        MVAL=din("MVAL", [128, NT], F32),
        VADD=din("VADD", [128, NT], F32),
        ETS=din("ETS", [128, NT, 4], F32R),
        SRCIDX=din("SRCIDX", [128, NT * 8], I16),
        DSTIDX=din("DSTIDX", [128, NT * 8], I16),
        SELIDX=din("SELIDX", [128, 8], I16),
        FCW=din("FCW", [128, NLAB], F32R),
        FCB=din("FCB", [1, NLAB], F32R),
        OUT=nc.dram_tensor("OUT", [B, NLAB], F32, kind="ExternalOutput"),
    )
    for l, inc in ((1, 6), (2, 8)):
        io[f"WQ{l}"] = din(f"WQ{l}", [inc * 128, 128], F32R)
        io[f"QB{l}"] = din(f"QB{l}", [128, 1], F32)
        io[f"WK{l}"] = din(f"WK{l}", [(inc + 1) * 128, 128], F32R)
        io[f"WV{l}"] = din(f"WV{l}", [(inc + 1) * 128, 128], F32R)
        io[f"WS{l}"] = din(f"WS{l}", [(inc + 1) * 128, 128], F32R)
        io[f"SB{l}"] = din(f"SB{l}", [128, 1], F32)
        io[f"E1WS{l}"] = din(f"E1WS{l}", [C1, 128], F32R)
        io[f"WRM4{l}"] = din(f"WRM4{l}", [C1, 4], F32R)

    with tile.TileContext(nc) as tc:
        with (
            tc.tile_pool(name="const", bufs=1) as cp,
            tc.tile_pool(name="ps", bufs=4, space="PSUM") as pp,
            tc.tile_pool(name="acc", bufs=2, space="PSUM") as ap_,
            tc.tile_pool(name="dram", bufs=1, space="DRAM") as dp,
        ):
            g = dict(io)
            g["pp"], g["ap_"], g["cp"], g["tc"] = pp, ap_, cp, tc

            # constants
            ident_f = cp.tile([128, 128], F32, tag="identf", name="identf")
            make_identity(nc, ident_f[:])
            g["ident"] = cp.tile([128, 128], F32R, tag="ident", name="ident")
            nc.vector.tensor_copy(out=g["ident"][:], in_=ident_f[:])
            iota_i = cp.tile([128, 128], I32)
            nc.gpsimd.iota(iota_i[:], pattern=[[1, 128]], base=0,
                           channel_multiplier=0)
            g["iota_f"] = cp.tile([128, 128], F32, tag="iotaf", name="iotaf")
            nc.vector.tensor_copy(out=g["iota_f"][:], in_=iota_i[:])
            ones_f = cp.tile([128, 128], F32, tag="onesf", name="onesf")
            nc.vector.memset(ones_f[:], 0.0)
            nc.vector.memset(ones_f[:1, :], 1.0)
            g["ones_blk"] = cp.tile([128, 128], F32R, tag="onesblk", name="onesblk")
            nc.vector.tensor_copy(out=g["ones_blk"][:], in_=ones_f[:])
            g["ones1"] = cp.tile([1, 128], F32R, tag="ones1", name="ones1")
            nc.vector.tensor_copy(out=g["ones1"][:], in_=ones_f[:1, :])

            # persistent DRAM
            g["AGIN0"] = dp.tile([D, NPC], F32R, tag="AGIN0", name="AGIN0")
            for _r in range(repeat):
                g[f"AGOUT0_{_r}"] = dp.tile([NCORES * D, NPC], F32R,
                                            tag=f"AGOUT0{_r}",
                                            name=f"AGOUT0{_r}",
                                            addr_space="Shared")
            g["AGIN1"] = dp.tile([128, N], F32R, tag="AGIN1", name="AGIN1")
            for _r in range(repeat):
                g[f"AGOUT1_{_r}"] = dp.tile([C1, N], F32R,
                                            tag=f"AGOUT1{_r}",
                                            name=f"AGOUT1{_r}",
                                            addr_space="Shared")
            g["KTAB"] = dp.tile([N, 128], BF16, tag="KTAB", name="KTAB")
            g["VTAB"] = dp.tile([N, 128], F32R, tag="VTAB", name="VTAB")
            g["PAYTAB"] = dp.tile([N, 256], BF16, tag="PAYTAB", name="PAYTAB")
            g["OUT2TAB"] = dp.tile([N, 128], F32R, tag="OUT2TAB", name="OUT2TAB")

            # edge arrays (persistent SBUF)
            for nm, shp, dt_ in (("DSTLOC", [128, NT], F32), ("TVAL", [128, NT], F32),
                                 ("MVAL", [128, NT], F32), ("VADD", [128, NT], F32),
                                 ("ETS", [128, NT, 4], F32R),
                                 ("SRCIDX", [128, NT * 8], I16),
                                 ("DSTIDX", [128, NT * 8], I16),
                                 ("SELIDX", [128, 8], I16)):
                t_ = cp.tile(shp, dt_, tag=f"sb{nm}", name=f"sb{nm}")
                nc.sync.dma_start(out=t_[:], in_=io[nm][:])
                g[f"sb{nm}"] = t_
            for nm, shp, dt_ in (("FCW", [128, NLAB], F32R), ("FCB", [1, NLAB], F32R)):
                t_ = cp.tile(shp, dt_, tag=f"sb{nm}", name=f"sb{nm}")
                nc.sync.dma_start(out=t_[:], in_=io[nm][:])
                g[f"sb{nm}"] = t_

            # hoisted weight staging (loop-invariant)
            _load_pool_weights(nc, tc, g)
            for l, inc in ((1, 6), (2, 8)):
                _load_layer_weights(nc, tc, g, l, inc)

            # ---- repeated body: segments in For_i, CCs straight-line ----
            with tc.For_i(0, repeat, 1):
                _pooling(nc, tc, g)
            for _r in range(repeat):
                nc.gpsimd.collective_compute(
                    "AllGather", AluOp.bypass,
                    replica_groups=[list(range(NCORES))],
                    ins=[g["AGIN0"].opt()], outs=[g[f"AGOUT0_{_r}"].opt()],
                )
            g["AGOUT0"] = g["AGOUT0_0"]
            with tc.For_i(0, repeat, 1):
                _layer(nc, tc, g, 1, 6, ntpb)
            for _r in range(repeat):
                nc.gpsimd.collective_compute(
                    "AllGather", AluOp.bypass,
                    replica_groups=[list(range(NCORES))],
                    ins=[g["AGIN1"].opt()], outs=[g[f"AGOUT1_{_r}"].opt()],
                )
            g["AGOUT1"] = g["AGOUT1_0"]
            with tc.For_i(0, repeat, 1):
                _layer(nc, tc, g, 2, 8, ntpb)
                _final(nc, tc, g)

    nc.compile()
    return nc


def _load_pool_weights(nc, tc, g):
    cp = g["cp"]
    w1s = cp.tile([128, 6, SPAN_H], F32R, tag="w1s", name="w1s")
    nc.sync.dma_start(out=w1s[:], in_=g["W1"][:].rearrange("(a p) h -> p a h", p=128))
    b1s = cp.tile([128, 4], F32, tag="b1s", name="b1s")
    nc.sync.dma_start(out=b1s[:], in_=g["B1C"][:])
    w2s = cp.tile([128, 4, 1], F32R, tag="w2s", name="w2s")
    nc.sync.dma_start(out=w2s[:], in_=g["W2"][:].rearrange("(a p) o -> p a o", p=128))
    masks = cp.tile([E, DPC, L], F32, tag="masks", name="masks")
    nc.sync.dma_start(out=masks[:], in_=g["MASKT"][:])
    g["w1s"], g["b1s"], g["w2s"], g["masks"] = w1s, b1s, w2s, masks


def _load_layer_weights(nc, tc, g, l, inc):
    cp, pp = g["cp"], g["pp"]
    ident = g["ident"]
    WQ = cp.tile([128, inc, 128], F32R, tag=f"WQ{l}s", name=f"WQ{l}s")
    nc.sync.dma_start(out=WQ[:], in_=g[f"WQ{l}"][:].rearrange("(a p) d -> p a d", p=128))
    WK = cp.tile([128, inc + 1, 128], F32R, tag=f"WK{l}s", name=f"WK{l}s")
    nc.sync.dma_start(out=WK[:], in_=g[f"WK{l}"][:].rearrange("(a p) d -> p a d", p=128))
    WV = cp.tile([128, inc + 1, 128], F32R, tag=f"WV{l}s", name=f"WV{l}s")
    nc.sync.dma_start(out=WV[:], in_=g[f"WV{l}"][:].rearrange("(a p) d -> p a d", p=128))
    WS = cp.tile([128, inc + 1, 128], F32R, tag=f"WS{l}s", name=f"WS{l}s")
    nc.sync.dma_start(out=WS[:], in_=g[f"WS{l}"][:].rearrange("(a p) d -> p a d", p=128))
    QB = cp.tile([128, 1], F32, tag=f"QB{l}s", name=f"QB{l}s")
    nc.sync.dma_start(out=QB[:], in_=g[f"QB{l}"][:])
    SB = cp.tile([128, 1], F32, tag=f"SB{l}s", name=f"SB{l}s")
    nc.sync.dma_start(out=SB[:], in_=g[f"SB{l}"][:])

    with tc.tile_pool(name=f"wtmp{l}", bufs=1) as wp:
        E1WS = wp.tile([128, 8, 128], F32R, tag="E1WS", name="E1WS")
        nc.sync.dma_start(out=E1WS[:], in_=g[f"E1WS{l}"][:].rearrange("(a p) d -> p a d", p=128))
        WRM4 = wp.tile([128, 8, 4], F32R, tag="WRM4", name="WRM4")
        nc.sync.dma_start(out=WRM4[:], in_=g[f"WRM4{l}"][:].rearrange("(a p) k -> p a k", p=128))
        # wvecs: rows of wv4 = [wc, wr, wm, junk]
        wrm = wp.tile([128, 8, 4], F32R, tag="wrm", name="wrm")
        nc.vector.tensor_copy(out=wrm[:], in_=WRM4[:])
        nc.vector.tensor_tensor(out=wrm[:, :, 0], in0=wrm[:, :, 0],
                                in1=wrm[:, :, 3], op=AluOp.add)
        wv_ps = pp.tile([128, 4], F32, space="PSUM", tag="ps", name="ps")
        for a in range(8):
            nc.tensor.matmul(out=wv_ps[:], lhsT=E1WS[:, a, :], rhs=wrm[:, a, :],
                             start=(a == 0), stop=(a == 7))
        wvecs = cp.tile([128, 4], F32R, tag=f"wvecs{l}", name=f"wvecs{l}")
        nc.vector.tensor_copy(out=wvecs[:], in_=wv_ps[:])
        wv4_ps = pp.tile([4, 128], F32R, space="PSUM", tag="ps", name="ps")
        nc.tensor.transpose(out=wv4_ps[:], in_=wvecs[:], identity=ident[:])
        wv4 = cp.tile([4, 128], F32R, tag=f"wv4{l}", name=f"wv4{l}")
        nc.vector.tensor_copy(out=wv4[:], in_=wv4_ps[:])
    g[f"WQ{l}s"], g[f"WK{l}s"], g[f"WV{l}s"], g[f"WS{l}s"] = WQ, WK, WV, WS
    g[f"QB{l}s"], g[f"SB{l}s"] = QB, SB
    g[f"wvecs{l}"], g[f"wv4{l}"] = wvecs, wv4


def _pooling(nc, tc, g):
    pp = g["pp"]
    ident, ones1 = g["ident"], g["ones1"]
    w1s, b1s, w2s, masks = g["w1s"], g["b1s"], g["w2s"], g["masks"]
    with tc.tile_pool(name="pool_ph", bufs=2) as sp, \
         tc.tile_pool(name="pool_ph1", bufs=1) as bp:
        eduT = bp.tile([128, 6, NPC], F32R, tag="eduT", name="eduT")

        for d in range(DPC):
            lhTd = sp.tile([128, 6, L], F32R, tag="lhTd", name="lhTd")
            nc.sync.dma_start(out=lhTd[:], in_=g["LHT"][:, d * L:(d + 1) * L]
                              .rearrange("(a p) t -> p a t", p=128))
            lhd = sp.tile([128, 4, D], F32R, tag="lhd", name="lhd")
            nc.sync.dma_start(out=lhd[:], in_=g["LH"][d * L:(d + 1) * L, :]
                              .rearrange("(a p) x -> p a x", p=128))

            h1 = sp.tile([128, 4, L], F32R, tag="h1", name="h1")
            for mc in range(4):
                h1_ps = pp.tile([128, L], F32, space="PSUM", tag="ps", name="ps")
                for a in range(6):
                    nc.tensor.matmul(out=h1_ps[:],
                                     lhsT=w1s[:, a, mc * 128:(mc + 1) * 128],
                                     rhs=lhTd[:, a, :], start=(a == 0), stop=(a == 5))
                nc.scalar.activation(out=h1[:, mc, :], in_=h1_ps[:], func=ActFn.Relu,
                                     bias=b1s[:, mc:mc + 1])
            att_ps = pp.tile([1, L], F32, space="PSUM", tag="ps", name="ps")
            for mc in range(4):
                nc.tensor.matmul(out=att_ps[:], lhsT=w2s[:, mc, :], rhs=h1[:, mc, :],
                                 start=(mc == 0), stop=(mc == 3))
            att_sb = sp.tile([1, L], F32R, tag="attsb", name="attsb")
            nc.vector.tensor_copy(out=att_sb[:], in_=att_ps[:])
            attb_ps = pp.tile([E, L], F32, space="PSUM", tag="ps", name="ps")
            nc.tensor.matmul(out=attb_ps[:], lhsT=ones1[:, :E], rhs=att_sb[:],
                             start=True, stop=True)
            mp = sp.tile([E, L], F32, tag="mp", name="mp")
            nc.vector.tensor_scalar(out=mp[:], in0=masks[:, d, :], scalar1=100000.0,
                                    scalar2=-100000.0, op0=AluOp.mult, op1=AluOp.add)
            logit = sp.tile([E, L], F32, tag="lgt", name="lgt")
            nc.vector.tensor_tensor(out=logit[:], in0=attb_ps[:], in1=mp[:],
                                    op=AluOp.add)
            ex = sp.tile([E, L], F32, tag="exl", name="exl")
            den = sp.tile([E, 1], F32, tag="denl", name="denl")
            nc.scalar.activation(out=ex[:], in_=logit[:], func=ActFn.Exp,
                                 accum_out=den[:])
            rcp = sp.tile([E, 1], F32, tag="rcpl", name="rcpl")
            nc.vector.reciprocal(out=rcp[:], in_=den[:])
            probs = sp.tile([E, L], F32R, tag="prb", name="prb")
            nc.vector.tensor_scalar_mul(out=probs[:], in0=ex[:], scalar1=rcp[:, :1])
            probsT = sp.tile([128, 4, E], F32R, tag="prbT", name="prbT")
            for lc in range(4):
                pt_ps = pp.tile([128, E], F32R, space="PSUM", tag="ps", name="ps")
                nc.tensor.transpose(out=pt_ps[:], in_=probs[:, lc * 128:(lc + 1) * 128],
                                    identity=ident[:E, :E])
                nc.vector.tensor_copy(out=probsT[:, lc, :], in_=pt_ps[:])
            for db in range(6):
                edu_ps = pp.tile([E, 128], F32, space="PSUM", tag="ps", name="ps")
                for lc in range(4):
                    nc.tensor.matmul(out=edu_ps[:], lhsT=probsT[:, lc, :],
                                     rhs=lhd[:, lc, db * 128:(db + 1) * 128],
                                     start=(lc == 0), stop=(lc == 3))
                edu_sb = sp.tile([E, 128], F32R, tag="edusb", name="edusb")
                nc.vector.tensor_copy(out=edu_sb[:], in_=edu_ps[:])
                et_ps = pp.tile([128, E], F32R, space="PSUM", tag="ps", name="ps")
                nc.tensor.transpose(out=et_ps[:], in_=edu_sb[:], identity=ident[:E, :E])
                nc.vector.tensor_copy(out=eduT[:, db, d * E:(d + 1) * E], in_=et_ps[:])
        nc.sync.dma_start(out=g["AGIN0"][:].rearrange("(a p) n -> p a n", p=128),
                          in_=eduT[:])


def _layer(nc, tc, g, l, inc, ntpb):
    pp, ap_ = g["pp"], g["ap_"]
    ident, iota_f = g["ident"], g["iota_f"]
    ones_blk, ones1 = g["ones_blk"], g["ones1"]
    KTAB, VTAB, PAYTAB = g["KTAB"], g["VTAB"], g["PAYTAB"]
    NTb8 = ntpb * 8
    WQ, WK, WV, WS = g[f"WQ{l}s"], g[f"WK{l}s"], g[f"WV{l}s"], g[f"WS{l}s"]
    QB, SB = g[f"QB{l}s"], g[f"SB{l}s"]
    wvecs, wv4 = g[f"wvecs{l}"], g[f"wv4{l}"]

    with tc.tile_pool(name=f"lp{l}", bufs=1) as bp, \
         tc.tile_pool(name=f"ls{l}", bufs=2) as sp, \
         tc.tile_pool(name=f"lg{l}", bufs=2) as gp:
        qT = bp.tile([128, N], F32R, tag="qT", name="qT")
        skipT = bp.tile([128, N], F32R, tag="skipT", name="skipT") if l == 1 else None
        s2nat = bp.tile([128, NBLK, 128], F32, tag="s2nat", name="s2nat") if l == 2 else None
        outT = bp.tile([128, N], F32R, tag="outT", name="outT") if l == 1 else None

        # ---- projections ----
        for b in range(NBLK):
            bs = slice(b * 128, (b + 1) * 128)
            xT = sp.tile([128, inc, 128], F32R, tag="xTblk", name="xTblk")
            if l == 1:
                r, h2 = b // 2, b % 2
                nc.sync.dma_start(
                    out=xT[:],
                    in_=g["AGOUT0"][r * D:(r + 1) * D, h2 * 128:(h2 + 1) * 128]
                        .rearrange("(a p) n -> p a n", p=128))
            else:
                nc.sync.dma_start(
                    out=xT[:],
                    in_=g["AGOUT1"][:, bs].rearrange("(a p) n -> p a n", p=128))
            q_ps = pp.tile([128, 128], F32, space="PSUM", tag="ps", name="ps")
            for a in range(inc):
                nc.tensor.matmul(out=q_ps[:], lhsT=WQ[:, a, :], rhs=xT[:, a, :],
                                 start=(a == 0), stop=(a == inc - 1))
            nc.vector.tensor_scalar_add(out=qT[:, bs], in0=q_ps[:], scalar1=QB[:, :1])
            if l == 1:
                s_ps = pp.tile([128, 128], F32, space="PSUM", tag="ps", name="ps")
                for a in range(inc):
                    nc.tensor.matmul(out=s_ps[:], lhsT=WS[:, a, :], rhs=xT[:, a, :],
                                     start=(a == 0), stop=(a == inc - 1))
                nc.vector.tensor_scalar_add(out=skipT[:, bs], in0=s_ps[:], scalar1=SB[:, :1])
            else:
                s_ps = pp.tile([128, 128], F32, space="PSUM", tag="ps", name="ps")
                for a in range(inc):
                    nc.tensor.matmul(out=s_ps[:], lhsT=xT[:, a, :], rhs=WS[:, a, :],
                                     start=(a == 0), stop=False)
                nc.tensor.matmul(out=s_ps[:], lhsT=ones_blk[:], rhs=WS[:, inc, :],
                                 start=False, stop=True)
                nc.vector.tensor_copy(out=s2nat[:, b, :], in_=s_ps[:])
            for nm, W_, tab, dt_ in (("k", WK, KTAB, BF16), ("v", WV, VTAB, F32R)):
                nat_ps = pp.tile([128, 128], F32, space="PSUM", tag="ps", name="ps")
                for a in range(inc):
                    nc.tensor.matmul(out=nat_ps[:], lhsT=xT[:, a, :], rhs=W_[:, a, :],
                                     start=(a == 0), stop=False)
                nc.tensor.matmul(out=nat_ps[:], lhsT=ones_blk[:], rhs=W_[:, inc, :],
                                 start=False, stop=True)
                stg = sp.tile([128, 128], dt_, tag=f"stg{nm}", name=f"stg{nm}")
                nc.vector.tensor_copy(out=stg[:], in_=nat_ps[:])
                nc.sync.dma_start(out=tab[bs, :], in_=stg[:])
            pay = sp.tile([128, 256], BF16, tag="paystg", name="paystg")
            qn_ps = pp.tile([128, 128], F32R, space="PSUM", tag="ps", name="ps")
            nc.tensor.transpose(out=qn_ps[:], in_=qT[:, bs], identity=ident[:])
            nc.vector.tensor_copy(out=pay[:, 0:128], in_=qn_ps[:])
            abc_ps = pp.tile([4, 128], F32, space="PSUM", tag="ps", name="ps")
            nc.tensor.matmul(out=abc_ps[:], lhsT=wvecs[:], rhs=qT[:, bs],
                             start=True, stop=True)
            abc_sb = sp.tile([4, 128], F32R, tag="abcsb", name="abcsb")
            nc.vector.tensor_copy(out=abc_sb[:], in_=abc_ps[:])
            abcT_ps = pp.tile([128, 4], F32R, space="PSUM", tag="ps", name="ps")
            nc.tensor.transpose(out=abcT_ps[:], in_=abc_sb[:], identity=ident[:4, :4])
            nc.vector.tensor_copy(out=pay[:, 128:132], in_=abcT_ps[:])
            nc.vector.memset(pay[:, 132:256], 0.0)
            nc.sync.dma_start(out=PAYTAB[bs, :], in_=pay[:])

        # ---- edges ----
        for b in range(NBLK):
            bs = slice(b * 128, (b + 1) * 128)
            ts_ = slice(b * ntpb, (b + 1) * ntpb)
            is_ = slice(b * NTb8, (b + 1) * NTb8)
            pay_b = gp.tile([128, ntpb, 256], BF16, tag="payb", name="payb")
            nc.gpsimd.dma_gather(pay_b[:], PAYTAB[bs, :], g["sbDSTIDX"][:, is_],
                                 ntpb * 128, ntpb * 128, 256, single_packet=False)
            ke_b = gp.tile([128, ntpb, 128], BF16, tag="keb", name="keb")
            nc.gpsimd.dma_gather(ke_b[:], KTAB[:], g["sbSRCIDX"][:, is_],
                                 ntpb * 128, ntpb * 128, 128, single_packet=False)
            ve_b = gp.tile([128, ntpb, 128], F32R, tag="veb", name="veb")
            nc.gpsimd.dma_gather(ve_b[:], VTAB[:], g["sbSRCIDX"][:, is_],
                                 ntpb * 128, ntpb * 128, 128, single_packet=False)
            prod = gp.tile([128, ntpb, 128], BF16, tag="prod", name="prod")
            nc.vector.tensor_tensor(out=prod[:], in0=pay_b[:, :, 0:128],
                                    in1=ke_b[:], op=AluOp.mult)
            dots = sp.tile([128, ntpb, 1], F32, tag="dots", name="dots")
            nc.vector.tensor_reduce(out=dots[:], in_=prod[:],
                                    axis=mybir.AxisListType.X, op=AluOp.add)
            abcf = sp.tile([128, ntpb, 4], F32, tag="abcf", name="abcf")
            nc.vector.tensor_copy(out=abcf[:], in_=pay_b[:, :, 128:132])
            tA = sp.tile([128, ntpb], F32, tag="tA", name="tA")
            nc.vector.tensor_tensor(out=tA[:], in0=g["sbTVAL"][:, ts_],
                                    in1=abcf[:, :, 1], op=AluOp.mult)
            mB = sp.tile([128, ntpb], F32, tag="mB", name="mB")
            nc.vector.tensor_tensor(out=mB[:], in0=g["sbMVAL"][:, ts_],
                                    in1=abcf[:, :, 2], op=AluOp.mult)
            lg = sp.tile([128, ntpb], F32, tag="lg", name="lg")
            nc.vector.tensor_tensor(out=lg[:], in0=dots[:, :, 0], in1=abcf[:, :, 0],
                                    op=AluOp.add)
            nc.vector.tensor_tensor(out=lg[:], in0=lg[:], in1=tA[:], op=AluOp.add)
            nc.vector.tensor_tensor(out=lg[:], in0=lg[:], in1=mB[:], op=AluOp.add)
            nc.vector.tensor_tensor(out=lg[:], in0=lg[:], in1=g["sbVADD"][:, ts_],
                                    op=AluOp.add)
            exb = sp.tile([128, ntpb], F32, tag="exb", name="exb")
            nc.scalar.activation(out=exb[:], in_=lg[:], func=ActFn.Exp, scale=ISD)

            agg_ps = ap_.tile([128, 128], F32, space="PSUM", tag="aggps", name="aggps")
            sums_ps = ap_.tile([4, 128], F32, space="PSUM", tag="sumsps", name="sumsps")
            for t in range(ntpb):
                s_t = sp.tile([128, 128], F32R, tag="stile", name="stile")
                nc.vector.tensor_scalar(
                    out=s_t[:], in0=iota_f[:],
                    scalar1=g["sbDSTLOC"][:, b * ntpb + t: b * ntpb + t + 1],
                    scalar2=exb[:, t:t + 1],
                    op0=AluOp.is_equal, op1=AluOp.mult)
                if l == 1:
                    nc.tensor.matmul(out=agg_ps[:], lhsT=ve_b[:, t, :], rhs=s_t[:],
                                     start=(t == 0), stop=False)
                else:
                    nc.tensor.matmul(out=agg_ps[:], lhsT=s_t[:], rhs=ve_b[:, t, :],
                                     start=(t == 0), stop=False)
                nc.tensor.matmul(out=sums_ps[:], lhsT=g["sbETS"][:, b * ntpb + t, :],
                                 rhs=s_t[:], start=(t == 0), stop=(t == ntpb - 1))
            sums_sb = sp.tile([4, 128], F32R, tag="sumssb", name="sumssb")
            nc.vector.tensor_copy(out=sums_sb[:], in_=sums_ps[:])
            if l == 1:
                nc.tensor.matmul(out=agg_ps[:], lhsT=wv4[:], rhs=sums_sb[:],
                                 start=False, stop=True)
            else:
                nc.tensor.matmul(out=agg_ps[:], lhsT=sums_sb[:], rhs=wv4[:],
                                 start=False, stop=True)
            deng = sp.tile([2, 128], F32, tag="deng", name="deng")
            nc.vector.tensor_scalar_max(out=deng[:], in0=sums_sb[:2, :], scalar1=1e-30)
            rcpr = sp.tile([2, 128], F32R, tag="rcpr", name="rcpr")
            with nc.allow_low_precision(reason="f32r recip feeds f32r matmul; ~1e-4 rel"):
                nc.vector.reciprocal(out=rcpr[:], in_=deng[:])
            if l == 1:
                rb_ps = pp.tile([128, 128], F32, space="PSUM", tag="ps", name="ps")
                nc.tensor.matmul(out=rb_ps[:], lhsT=ones1[:], rhs=rcpr[:1, :],
                                 start=True, stop=True)
                rb_sb = sp.tile([128, 128], F32, tag="rbsb", name="rbsb")
                nc.vector.tensor_copy(out=rb_sb[:], in_=rb_ps[:])
                tmp = sp.tile([128, 128], F32, tag="cmb", name="cmb")
                nc.vector.tensor_tensor(out=tmp[:], in0=agg_ps[:], in1=rb_sb[:],
                                        op=AluOp.mult)
                nc.vector.tensor_tensor(out=outT[:, bs], in0=tmp[:], in1=skipT[:, bs],
                                        op=AluOp.add)
            else:
                rc_ps = pp.tile([128, 2], F32R, space="PSUM", tag="ps", name="ps")
                nc.tensor.transpose(out=rc_ps[:], in_=rcpr[:], identity=ident[:2, :2])
                rc_sb = sp.tile([128, 1], F32, tag="rcsb", name="rcsb")
                nc.vector.tensor_copy(out=rc_sb[:], in_=rc_ps[:, :1])
                tmp = sp.tile([128, 128], F32, tag="cmb", name="cmb")
                nc.vector.tensor_scalar_mul(out=tmp[:], in0=agg_ps[:],
                                            scalar1=rc_sb[:, :1])
                o2 = sp.tile([128, 128], F32R, tag="o2", name="o2")
                nc.vector.tensor_tensor(out=o2[:], in0=tmp[:], in1=s2nat[:, b, :],
                                        op=AluOp.add)
                nc.sync.dma_start(out=g["OUT2TAB"][bs, :], in_=o2[:])
        if l == 1:
            nc.sync.dma_start(out=g["AGIN1"][:], in_=outT[:])


def _final(nc, tc, g):
    pp = g["pp"]
    with tc.tile_pool(name="fin", bufs=1) as sp:
        sel = sp.tile([128, 1, 128], F32R, tag="sel", name="sel")
        nc.gpsimd.dma_gather(sel[:], g["OUT2TAB"][:], g["sbSELIDX"][:], 128, 128, 128)
        selT_ps = pp.tile([128, 128], F32R, space="PSUM", tag="ps", name="ps")
        nc.tensor.transpose(out=selT_ps[:], in_=sel[:, 0, :], identity=g["ident"][:])
        selT_sb = sp.tile([128, 128], F32R, tag="selTsb", name="selTsb")
        nc.vector.tensor_copy(out=selT_sb[:], in_=selT_ps[:])
        fc_ps = pp.tile([128, NLAB], F32, space="PSUM", tag="ps", name="ps")
        nc.tensor.matmul(out=fc_ps[:], lhsT=selT_sb[:], rhs=g["sbFCW"][:],
                         start=True, stop=False)
        nc.tensor.matmul(out=fc_ps[:], lhsT=g["ones1"][:], rhs=g["sbFCB"][:],
                         start=False, stop=True)
        fc_sb = sp.tile([128, NLAB], F32, tag="fcsb", name="fcsb")
        nc.vector.tensor_copy(out=fc_sb[:], in_=fc_ps[:])
        nc.sync.dma_start(out=g["OUT"][:], in_=fc_sb[:B, :])


# --------------------------------------------------------------------------
# host side
# --------------------------------------------------------------------------

_CACHE = {}


def get_program(ntpb, repeat=1):
    key = (ntpb, repeat)
    if key not in _CACHE:
        _CACHE[key] = build_program(ntpb, repeat)
    return _CACHE[key]


def prepare_in_maps(inputs):
    inp = {k: np.asarray(v) for k, v in inputs.items()}
    lh = inp["last_hidden"].astype(np.float32)
    mask = inp["batch_edu_mask"].astype(np.float32)
    lens = inp["edu_lengths"].astype(np.int64)
    edges = inp["batch_edge"].astype(np.int64)

    cnt = np.bincount(edges[1] // 128, minlength=NBLK)
    ntpb = max(18, int(math.ceil(cnt.max() / 128)))

    ed = _edge_prep(edges, inp["batch_edge_type"], inp["batch_is_main"], ntpb)
    selidx = (np.arange(B) * E + (lens - 1)).astype(np.int16)
    ed["SELIDX"] = _wrap_idx(selidx, pad_to=128)

    b1 = inp["b1"].astype(np.float32)
    common = dict(
        W1=inp["W1"].astype(np.float32),
        B1C=np.ascontiguousarray(b1.reshape(4, 128).T),
        W2=inp["W2"].astype(np.float32),
        **ed,
    )

    in_maps = []
    for c in range(NCORES):
        im = dict(common)
        lhc = lh[c * DPC:(c + 1) * DPC].reshape(DPC * L, D)
        im["LH"] = np.ascontiguousarray(lhc)
        im["LHT"] = np.ascontiguousarray(lhc.T)
        im["MASKT"] = np.ascontiguousarray(
            mask[c * DPC:(c + 1) * DPC].transpose(1, 0, 2))
        hs = slice(c * 128, (c + 1) * 128)
        for l, p in ((1, "1"), (2, "2")):
            im[f"WQ{l}"] = np.ascontiguousarray(inp[f"q{p}W"].astype(np.float32)[:, hs])
            im[f"QB{l}"] = np.ascontiguousarray(
                inp[f"q{p}b"].astype(np.float32)[hs].reshape(128, 1))
            for nm, wk, bk in (("WK", f"k{p}W", f"k{p}b"),
                               ("WV", f"v{p}W", f"v{p}b"),
                               ("WS", f"s{p}W", f"s{p}b")):
                w = inp[wk].astype(np.float32)[:, hs]
                ext = np.zeros((w.shape[0] + 128, 128), np.float32)
                ext[:w.shape[0]] = w
                ext[w.shape[0]] = inp[bk].astype(np.float32)[hs]
                im[f"{nm}{l}"] = ext
            im[f"SB{l}"] = np.ascontiguousarray(
                inp[f"s{p}b"].astype(np.float32)[hs].reshape(128, 1))
            im[f"E1WS{l}"] = np.ascontiguousarray(
                inp[f"e{p}W"].astype(np.float32)[:, hs])
            wr = inp[f"Wr{p}"].astype(np.float32).reshape(-1)
            wm = inp[f"Wm{p}"].astype(np.float32).reshape(-1)
            br = inp[f"br{p}"].astype(np.float32)
            bm = inp[f"bm{p}"].astype(np.float32)
            im[f"WRM4{l}"] = np.ascontiguousarray(
                np.stack([br, wr, wm, bm], axis=1).astype(np.float32))
        im["FCW"] = np.ascontiguousarray(inp["fcW"].astype(np.float32)[hs, :])
        fcb = inp["fcb"].astype(np.float32).reshape(1, NLAB)
        im["FCB"] = fcb if c == 0 else np.zeros_like(fcb)
        in_maps.append(im)
    return in_maps, ntpb


def run(inputs, repeat=1):
    in_maps, ntpb = prepare_in_maps(inputs)
    nc = get_program(ntpb, repeat)
    res = run_bass_kernel_spmd(nc, in_maps, list(range(NCORES)))
    out = np.zeros((B, NLAB), np.float64)
    for c in range(NCORES):
        out += res.results[c]["OUT"].astype(np.float64)
    return out.astype(np.float32)


def kernel(**inputs) -> np.ndarray:
    return run(inputs)
